# revision 1
# baseline (speedup 1.0000x reference)
"""Trainium2 Bass kernel for the BAHDANAU+ group-recommendation model.

kernel(**inputs) takes the complete (unsharded) numpy inputs, shards the
131072-query batch across 8 NeuronCores (data-parallel, tables replicated),
runs the Bass kernel SPMD, and returns the full [B, 1] float32 output.

Per-core dataflow (16384 rows = 128 tiles of 128 rows, gathered in batches
of GT tiles per indirect DMA to amortize the ~1us SWDGE issue cost):
  - two-level gather on device: group ids -> members rows -> 3 user_emb rows
    each, item ids -> fused item table row (item_emb || genres)
  - batch-major DVE compute with native ops only: attention logits via
    mult+reduce dot products, weighted member sum via free-dim-broadcast
    multiplies, 2-layer MLP the same way, sigmoid on the scalar engine.
Small weights arrive pre-broadcast across partitions from the host, so no
partition-broadcast DMAs are needed on device.
"""

import sys

sys.path.insert(0, "/opt/trn_rl_repo")

from contextlib import ExitStack

import numpy as np

import concourse.bacc as bacc
import concourse.bass as bass
import concourse.tile as tile
from concourse import mybir
from concourse.bass_utils import run_bass_kernel_spmd

N_CORES = 8
P = 128
EMB = 32
B = 131_072
NUM_USERS = 1_000_000
NUM_ITEMS = 100_000
NUM_GROUPS = 500_000
ROWS_PER_CORE = B // N_CORES
GT = 1  # tiles per gather batch (multi-col idx indirect is broken on HW)

F32 = mybir.dt.float32
I32 = mybir.dt.int32
MULT = mybir.AluOpType.mult
ADD = mybir.AluOpType.add
AXX = mybir.AxisListType.X


def build(nrows):
    """Build the per-core Bass program for `nrows` rows (must be /128)."""
    assert nrows % P == 0
    ntiles = nrows // P
    gt = min(GT, ntiles)
    assert ntiles % gt == 0

    nc = bacc.Bacc(
        "TRN2", target_bir_lowering=False, debug=False, enable_asserts=False
    )

    grp = nc.dram_tensor("grp_idx", [P, ntiles], I32, kind="ExternalInput")
    itm = nc.dram_tensor("item_idx", [P, ntiles], I32, kind="ExternalInput")
    members = nc.dram_tensor("members_t", [NUM_GROUPS, 4], I32, kind="ExternalInput")
    user_tab = nc.dram_tensor("user_tab", [NUM_USERS, EMB], F32, kind="ExternalInput")
    item_tab = nc.dram_tensor("item_tab", [NUM_ITEMS, EMB], F32, kind="ExternalInput")
    attn_bcd = nc.dram_tensor("attn_bc", [P, 3, 4 * EMB], F32, kind="ExternalInput")
    attnb_bcd = nc.dram_tensor("attnb_bc", [P, 3], F32, kind="ExternalInput")
    w1_bcd = nc.dram_tensor("w1_bc", [P, 8, 3 * EMB], F32, kind="ExternalInput")
    b1_bcd = nc.dram_tensor("b1_bc", [P, 8], F32, kind="ExternalInput")
    w2_bcd = nc.dram_tensor("w2_bc", [P, 8], F32, kind="ExternalInput")
    b2_bcd = nc.dram_tensor("b2_bc", [P, 1], F32, kind="ExternalInput")
    y_out = nc.dram_tensor("y_out", [P, ntiles], F32, kind="ExternalOutput")

    with tile.TileContext(nc) as tc, ExitStack() as ctx:
        singles = ctx.enter_context(tc.tile_pool(name="singles", bufs=1))
        gathp = ctx.enter_context(tc.tile_pool(name="gathp", bufs=3))
        gip = ctx.enter_context(tc.tile_pool(name="gip", bufs=6))
        smal = ctx.enter_context(tc.tile_pool(name="smal", bufs=6))

        attn_bc = singles.tile([P, 3, 4 * EMB], F32)
        nc.sync.dma_start(out=attn_bc[:], in_=attn_bcd.ap())
        attnb_bc = singles.tile([P, 3], F32)
        nc.sync.dma_start(out=attnb_bc[:], in_=attnb_bcd.ap())
        w1_bc = singles.tile([P, 8, 3 * EMB], F32)
        nc.sync.dma_start(out=w1_bc[:], in_=w1_bcd.ap())
        b1_bc = singles.tile([P, 8], F32)
        nc.sync.dma_start(out=b1_bc[:], in_=b1_bcd.ap())
        w2_bc = singles.tile([P, 8], F32)
        nc.sync.dma_start(out=w2_bc[:], in_=w2_bcd.ap())
        b2_bc = singles.tile([P, 1], F32)
        nc.sync.dma_start(out=b2_bc[:], in_=b2_bcd.ap())

        grp_all = singles.tile([P, ntiles], I32)
        nc.sync.dma_start(out=grp_all[:], in_=grp.ap())
        itm_all = singles.tile([P, ntiles], I32)
        nc.sync.dma_start(out=itm_all[:], in_=itm.ap())

        ypre = singles.tile([P, ntiles], F32)

        for s in range(ntiles):
            t0 = s
            # ---- per-tile gathers (2-D out APs, HW-proven pattern) -----
            mem_idx = gathp.tile([P, 4], I32, tag="mem_idx")
            nc.gpsimd.indirect_dma_start(
                out=mem_idx[:],
                out_offset=None,
                in_=members.ap(),
                in_offset=bass.IndirectOffsetOnAxis(
                    ap=grp_all[:, t0 : t0 + 1], axis=0
                ),
            )
            gbig = gathp.tile([P, 4 * EMB], F32, tag="gbig")
            for k in range(3):
                nc.gpsimd.indirect_dma_start(
                    out=gbig[:, k * EMB : (k + 1) * EMB],
                    out_offset=None,
                    in_=user_tab.ap(),
                    in_offset=bass.IndirectOffsetOnAxis(
                        ap=mem_idx[:, k : k + 1], axis=0
                    ),
                )
            nc.gpsimd.indirect_dma_start(
                out=gbig[:, 3 * EMB : 4 * EMB],
                out_offset=None,
                in_=item_tab.ap(),
                in_offset=bass.IndirectOffsetOnAxis(
                    ap=itm_all[:, t0 : t0 + 1], axis=0
                ),
            )

            for ti in range(1):
                t = t0 + ti
                gi = gbig[:]
                # ---- attention logits: at[:, j] = gi . attn_W[:, j] + b
                at = smal.tile([P, 3], F32, tag="at")
                tmp = gip.tile([P, 3, 4 * EMB], F32, tag="tmp")
                nc.vector.tensor_tensor(
                    out=tmp[:],
                    in0=gi.unsqueeze(1).to_broadcast([P, 3, 4 * EMB]),
                    in1=attn_bc[:],
                    op=MULT,
                )
                nc.vector.tensor_reduce(out=at[:], in_=tmp[:], axis=AXX, op=ADD)
                nc.vector.tensor_tensor(
                    out=at[:], in0=at[:], in1=attnb_bc[:], op=ADD
                )

                # ---- g = sum_k at[:, k] * mem_k ------------------------
                newt = gip.tile([P, 3 * EMB], F32, tag="newt")
                gm = smal.tile([P, 3, EMB], F32, tag="gm")
                nc.vector.tensor_tensor(
                    out=gm[:],
                    in0=gi[:, 0 : 3 * EMB].rearrange("p (k d) -> p k d", k=3),
                    in1=at[:].unsqueeze(2).to_broadcast([P, 3, EMB]),
                    op=MULT,
                )
                nc.vector.tensor_reduce(
                    out=newt[:, EMB : 2 * EMB],
                    in_=gm[:].rearrange("p k d -> p d k"),
                    axis=AXX,
                    op=ADD,
                )
                # new = [g * it, g, it]
                nc.vector.tensor_tensor(
                    out=newt[:, 0:EMB],
                    in0=newt[:, EMB : 2 * EMB],
                    in1=gi[:, 3 * EMB : 4 * EMB],
                    op=MULT,
                )
                nc.vector.tensor_copy(
                    out=newt[:, 2 * EMB : 3 * EMB], in_=gi[:, 3 * EMB : 4 * EMB]
                )

                # ---- MLP: h = relu(new @ W1 + b1) ----------------------
                h = smal.tile([P, 8], F32, tag="h")
                tmp2 = gip.tile([P, 8, 3 * EMB], F32, tag="tmp2")
                nc.vector.tensor_tensor(
                    out=tmp2[:],
                    in0=newt[:].unsqueeze(1).to_broadcast([P, 8, 3 * EMB]),
                    in1=w1_bc[:],
                    op=MULT,
                )
                nc.vector.tensor_reduce(out=h[:], in_=tmp2[:], axis=AXX, op=ADD)
                nc.vector.tensor_tensor(out=h[:], in0=h[:], in1=b1_bc[:], op=ADD)
                nc.vector.tensor_scalar_max(h[:], h[:], 0.0)

                # ---- y = h @ W2 (bias+sigmoid applied at the end) ------
                tmp3 = smal.tile([P, 8], F32, tag="tmp3")
                nc.vector.tensor_tensor(
                    out=tmp3[:], in0=h[:], in1=w2_bc[:], op=MULT
                )
                nc.vector.tensor_reduce(
                    out=ypre[:, t : t + 1], in_=tmp3[:], axis=AXX, op=ADD
                )

        ypre2 = singles.tile([P, ntiles], F32)
        nc.vector.tensor_tensor(
            out=ypre2[:],
            in0=ypre[:],
            in1=b2_bc[:, 0:1].to_broadcast([P, ntiles]),
            op=ADD,
        )
        ysig = singles.tile([P, ntiles], F32)
        nc.scalar.activation(
            out=ysig[:], in_=ypre2[:], func=mybir.ActivationFunctionType.Sigmoid
        )
        nc.sync.dma_start(out=y_out.ap(), in_=ysig[:])

    nc.compile()
    return nc


def prep_host_inputs(inputs):
    """Cast/convert the full problem inputs into device-table layouts."""
    grp = np.asarray(inputs["group_inputs"]).astype(np.int32).reshape(-1)
    itm = np.asarray(inputs["item_inputs"]).astype(np.int32).reshape(-1)
    members = np.asarray(inputs["members"]).astype(np.int32)
    members4 = np.zeros((members.shape[0], 4), np.int32)
    members4[:, :3] = members
    user_tab = np.ascontiguousarray(np.asarray(inputs["user_emb"], np.float32))
    item_tab = np.ascontiguousarray(
        np.concatenate(
            [
                np.asarray(inputs["item_emb"], np.float32),
                np.asarray(inputs["genres"], np.float32),
            ],
            axis=1,
        )
    )
    attn_W = np.asarray(inputs["attn_W"], np.float32)  # [128, 3]
    attn_b = np.asarray(inputs["attn_b"], np.float32)  # [3]
    w1 = np.asarray(inputs["pred_W1"], np.float32)  # [96, 8]
    b1 = np.asarray(inputs["pred_b1"], np.float32)  # [8]
    w2 = np.asarray(inputs["pred_W2"], np.float32)  # [8, 1]
    b2 = np.asarray(inputs["pred_b2"], np.float32)  # [1]
    ones = np.ones((P, 1, 1), np.float32)
    w = {
        "attn_bc": np.ascontiguousarray(ones * attn_W.T[None, :, :]),
        "attnb_bc": np.ascontiguousarray(np.tile(attn_b[None, :], (P, 1))),
        "w1_bc": np.ascontiguousarray(ones * w1.T[None, :, :]),
        "b1_bc": np.ascontiguousarray(np.tile(b1[None, :], (P, 1))),
        "w2_bc": np.ascontiguousarray(np.tile(w2[:, 0][None, :], (P, 1))),
        "b2_bc": np.ascontiguousarray(np.tile(b2[None, :], (P, 1))),
    }
    return grp, itm, members4, user_tab, item_tab, w


def make_in_maps(grp, itm, members4, user_tab, item_tab, w, nrows, n_cores):
    ntiles = nrows // P
    in_maps = []
    for c in range(n_cores):
        sl = slice(c * nrows, (c + 1) * nrows)
        in_maps.append(
            {
                "grp_idx": np.ascontiguousarray(grp[sl].reshape(P, ntiles)),
                "item_idx": np.ascontiguousarray(itm[sl].reshape(P, ntiles)),
                "members_t": members4,
                "user_tab": user_tab,
                "item_tab": item_tab,
                **w,
            }
        )
    return in_maps


_NC_CACHE = {}


def kernel(**inputs) -> np.ndarray:
    grp, itm, members4, user_tab, item_tab, w = prep_host_inputs(inputs)
    if ROWS_PER_CORE not in _NC_CACHE:
        _NC_CACHE[ROWS_PER_CORE] = build(ROWS_PER_CORE)
    nc = _NC_CACHE[ROWS_PER_CORE]
    in_maps = make_in_maps(
        grp, itm, members4, user_tab, item_tab, w, ROWS_PER_CORE, N_CORES
    )
    res = run_bass_kernel_spmd(nc, in_maps, core_ids=list(range(N_CORES)))
    outs = [res.results[c]["y_out"].reshape(ROWS_PER_CORE) for c in range(N_CORES)]
    return np.concatenate(outs).reshape(B, 1).astype(np.float32)

